# revision 15
# baseline (speedup 1.0000x reference)
"""Trainium2 Bass kernel for CustomSelfAttention (B=4, S=2048, D=1024, H=16).

Sharding: 8 cores = batch (4) x head-half (2). Each core projects Q/K/V for
its 8 heads over the full 2048-token sequence, runs attention for those
heads, and computes a PARTIAL output projection (contraction over its 512
model dims). The host sums the two partials per batch during unshard.

Device layout notes:
  - Host pre-transposes x -> x^T [D, S] and mod -> mod^T [S_k, S_q] (bf16).
    Query-padding mask and the 1/sqrt(hd) scale are folded into the mask
    modifier / Wq on the host. bv is folded into the output bias via
    bo' = bv_half @ Wo_half (+ bo on the even core only).
  - Energy is computed transposed (e^T[k, q]) so softmax normalization
    sums arrive via an appended ones-column in the V matmul (M=65), and
    exp() output feeds the attn@V matmul with no transposes.
  - Softmax skips max-subtraction: |energy*mod| <= ~8, exp() is safe.
  - Emission drip-feeds projection/output-chunk work into the attention
    loop so the tensor engine fills the slack while vector (mod-multiply)
    and scalar (exp) run; avoids the long tensor-only warmup.
"""

import numpy as np
import ml_dtypes

B, S, D, H = 4, 2048, 1024, 16
HD = D // H          # 64
HL = H // 2          # 8 local heads per core
DL = HL * HD         # 512 local dims
N_CORES = 8
NDC = D // 128       # 8 dim chunks (full D)
NLC = DL // 128      # 4 local dim chunks
NKC = S // 128       # 16 key chunks
NQB = S // 512       # 4 query blocks
BF = ml_dtypes.bfloat16

_CACHE = {}


def _emit(nc, tc, mybir, io):
    f32 = mybir.dt.float32
    bf = mybir.dt.bfloat16
    Exp = mybir.ActivationFunctionType.Exp
    Copy = mybir.ActivationFunctionType.Copy
    Ident = mybir.ActivationFunctionType.Identity
    mult = mybir.AluOpType.mult
    xT, modT, wq, wk, wv, wo, bqd, bkd, bod, out = io

    from contextlib import ExitStack
    with ExitStack() as _es:
        def _pool(name, bufs, **kw):
            return _es.enter_context(tc.tile_pool(name=name, bufs=bufs, **kw))
        Pv = _pool("pv", NKC)
        Pm = _pool("pmod", 26)
        Pq = _pool("pqT", NLC)
        Pk = _pool("pkT", NLC)
        Pa = _pool("pao", NLC)
        Px = _pool("pxT", NDC)
        Pwq = _pool("pwq", NDC)
        Pwk = _pool("pwk", NDC)
        Pwv = _pool("pwv", NDC)
        Pwo = _pool("pwo", NLC)
        Pe = _pool("pesb", 2)
        Pex = _pool("pex", 2)
        Pbc = _pool("pbc", 1)
        Prs = _pool("prs", 1)
        Po = _pool("pout", 2)
        Pc = _pool("pmisc", 1)

        # ---- constants & weights ----
        bq_sb = Pc.tile([128, NLC], f32, tag="bq")
        bk_sb = Pc.tile([128, NLC], f32, tag="bk")
        nc.sync.dma_start(out=bq_sb[:], in_=bqd[:].rearrange("(c p) -> p c", p=128))
        nc.sync.dma_start(out=bk_sb[:], in_=bkd[:].rearrange("(c p) -> p c", p=128))
        ones_sb = Pc.tile([1, 128], bf, tag="ones")
        nc.gpsimd.memset(ones_sb[:], 1.0)
        bo_sb = Pc.tile([1, D], bf, tag="bo")
        nc.sync.dma_start(out=bo_sb[:], in_=bod[:])

        x_sb = []
        for dc in range(NDC):
            t = Px.tile([128, S], bf, tag="xT", name=f"xT{dc}")
            nc.sync.dma_start(out=t[:], in_=xT[dc * 128:(dc + 1) * 128, :])
            x_sb.append(t)
        wq_sb, wk_sb, wv_sb = [], [], []
        for dc in range(NDC):
            t = Pwq.tile([128, DL], bf, tag="wq", name=f"wq{dc}")
            nc.sync.dma_start(out=t[:], in_=wq[dc * 128:(dc + 1) * 128, :])
            wq_sb.append(t)
            t = Pwk.tile([128, DL], bf, tag="wk", name=f"wk{dc}")
            nc.sync.dma_start(out=t[:], in_=wk[dc * 128:(dc + 1) * 128, :])
            wk_sb.append(t)
            t = Pwv.tile([128, DL], bf, tag="wv", name=f"wv{dc}")
            nc.sync.dma_start(out=t[:], in_=wv[dc * 128:(dc + 1) * 128, :])
            wv_sb.append(t)
        wo_sb = []
        for dc in range(NLC):
            t = Pwo.tile([128, D], bf, tag="wo", name=f"wo{dc}")
            nc.sync.dma_start(out=t[:], in_=wo[dc * 128:(dc + 1) * 128, :])
            wo_sb.append(t)

        # mod tiles: [128 keys, 512 queries] per (qb, kc); qb 2,3 reuse
        # qb 0,1 slots (32-buf pool).
        mod_sb = {}
        for qb in range(NQB):
            for kc in range(NKC):
                m = Pm.tile([128, 512], bf, tag="mod", name=f"mod{qb}_{kc}")
                nc.sync.dma_start(
                    out=m[:],
                    in_=modT[kc * 128:(kc + 1) * 128, qb * 512:(qb + 1) * 512])
                mod_sb[(qb, kc)] = m

        v_sb = [Pv.tile([128, HL * 66 + 64], bf, tag="v", name=f"v{i}")
                for i in range(NKC)]
        qT = [Pq.tile([128, S], bf, tag="qT", name=f"qT{i}") for i in range(NLC)]
        kT = [Pk.tile([128, S], bf, tag="kT", name=f"kT{i}") for i in range(NLC)]
        aoT = [Pa.tile([128, S], bf, tag="aoT", name=f"aoT{i}")
               for i in range(NLC)]

        PSB = _pool("psm", 2, space="PSUM")
        PSe = _pool("pse", 2, space="PSUM")
        PSo = _pool("pso", 1, space="PSUM")
        if True:

            def q_chunk(hp, blk):
                # qT[hp][:, blk*512:+512] = (x @ Wq-half)^T chunk + bias
                ps = PSB.tile([128, 512], f32, tag="psb", name=f"psq{hp}_{blk}")
                for kc in range(NDC):
                    nc.tensor.matmul(
                        out=ps[:],
                        lhsT=wq_sb[kc][:, hp * 128:(hp + 1) * 128],
                        rhs=x_sb[kc][:, blk * 512:(blk + 1) * 512],
                        start=(kc == 0), stop=(kc == NDC - 1))
                nc.scalar.activation(
                    out=qT[hp][:, blk * 512:(blk + 1) * 512],
                    in_=ps[:], func=Ident, bias=bq_sb[:, hp:hp + 1])

            def k_chunk(hp, blk):
                ps = PSB.tile([128, 512], f32, tag="psb", name=f"psk{hp}_{blk}")
                for kc in range(NDC):
                    nc.tensor.matmul(
                        out=ps[:],
                        lhsT=wk_sb[kc][:, hp * 128:(hp + 1) * 128],
                        rhs=x_sb[kc][:, blk * 512:(blk + 1) * 512],
                        start=(kc == 0), stop=(kc == NDC - 1))
                nc.scalar.activation(
                    out=kT[hp][:, blk * 512:(blk + 1) * 512],
                    in_=ps[:], func=Ident, bias=bk_sb[:, hp:hp + 1])

            def v_chunk(sc):
                # v rows for seq chunk sc, all 8 local heads (+ ones col)
                ps = PSB.tile([128, 512], f32, tag="psb", name=f"psv{sc}")
                for dc in range(NDC):
                    nc.tensor.matmul(
                        out=ps[:],
                        lhsT=x_sb[dc][:, sc * 128:(sc + 1) * 128],
                        rhs=wv_sb[dc][:],
                        start=(dc == 0), stop=(dc == NDC - 1))
                v3 = v_sb[sc][:, 0:HL * 66].rearrange("p (h d) -> p h d", d=66)
                nc.gpsimd.memset(v3[:, :, 64:65], 1.0)
                nc.gpsimd.memset(v_sb[sc][:, HL * 66:], 0.0)
                nc.scalar.activation(
                    out=v3[:, :, 0:64],
                    in_=ps[:].rearrange("p (h d) -> p h d", d=64),
                    func=Copy)

            def out_chunk(sc, blk):
                # partial out rows sc*128..+128, cols blk*512..+512
                pf = PSB.tile([128, 512], f32, tag="psb", name=f"pf{sc}_{blk}")
                nc.tensor.matmul(
                    out=pf[:], lhsT=ones_sb[:],
                    rhs=bo_sb[:, blk * 512:(blk + 1) * 512],
                    start=True, stop=False)
                for dc in range(NLC):
                    nc.tensor.matmul(
                        out=pf[:],
                        lhsT=aoT[dc][:, sc * 128:(sc + 1) * 128],
                        rhs=wo_sb[dc][:, blk * 512:(blk + 1) * 512],
                        start=False, stop=(dc == NLC - 1))
                osb = Po.tile([128, 512], f32, tag="osb", name=f"osb{sc}_{blk}")
                nc.scalar.activation(out=osb[:], in_=pf[:], func=Copy)
                nc.sync.dma_start(
                    out=out[sc * 128:(sc + 1) * 128,
                            blk * 512:(blk + 1) * 512],
                    in_=osb[:])

            # drip-feed queue: thunks of ~2us tensor work popped inside the
            # attention loop at a controlled per-kpp rate. Order matters: a
            # thunk must be emitted no later than the kpp whose energy/attnV
            # matmuls consume its output (engine queues run in emission
            # order, so a consumer emitted before its producer deadlocks).
            work = []

            def attention(qb, hp, pops=(1, 1, 1, 1)):
                o_ps = PSo.tile([128, 1024], f32, tag="o",
                                name=f"o_{qb}_{hp}")
                for kpp in range(NKC // 4):
                    for _ in range(pops[kpp]):
                        if work:
                            work.pop(0)()
                    e4 = Pe.tile([128, 4096], bf, tag="e",
                                 name=f"e{qb}_{hp}_{kpp}")
                    ex4 = Pex.tile([128, 4096], bf, tag="ex",
                                   name=f"ex{qb}_{hp}_{kpp}")
                    for jj in range(4):
                        kc = kpp * 4 + jj
                        pe_ps = PSe.tile([128, 1024], f32, tag="pe",
                                         name=f"pe{qb}_{hp}_{kc}")
                        for i in range(2):
                            nc.tensor.matmul(
                                out=pe_ps[:, i * 512:(i + 1) * 512],
                                lhsT=kT[hp][i * 64:(i + 1) * 64,
                                            kc * 128:(kc + 1) * 128],
                                rhs=qT[hp][i * 64:(i + 1) * 64,
                                           qb * 512:(qb + 1) * 512],
                                start=True, stop=True)
                        rep = (mod_sb[(qb, kc)][:, 0:512]
                               .unsqueeze(1).broadcast_to((128, 2, 512)))
                        nc.vector.scalar_tensor_tensor(
                            out=e4[:, jj * 1024:(jj + 1) * 1024]
                                .rearrange("p (r c) -> p r c", r=2),
                            in0=pe_ps[:].rearrange("p (r c) -> p r c", r=2),
                            scalar=1.0, in1=rep, op0=mult, op1=mult)
                    nc.scalar.activation(out=ex4[:], in_=e4[:], func=Exp)
                    for jj in range(4):
                        kc = kpp * 4 + jj
                        for i in range(2):
                            h = hp * 2 + i
                            nc.tensor.matmul(
                                out=o_ps[:, i * 512:(i + 1) * 512],
                                lhsT=v_sb[kc][:, h * 66:h * 66 + 128],
                                rhs=ex4[:, jj * 1024 + i * 512:
                                        jj * 1024 + (i + 1) * 512],
                                start=(kc == 0), stop=(kc == NKC - 1))
                # normalize: sums live in psum row 64 (ones column). recip
                # reads psum directly; partition_broadcast (GpSimd, ~1us)
                # hides behind the other head's work.
                su = Prs.tile([1, 1024], f32, tag="su",
                              name=f"su{qb}_{hp}")
                rc = Prs.tile([1, 1024], f32, tag="rc",
                              name=f"rc{qb}_{hp}")
                bc = Pbc.tile([128, 1024], f32, tag="bc",
                              name=f"bc{qb}_{hp}")
                nc.scalar.activation(out=su[0:1, :],
                                     in_=o_ps[64:65, :], func=Copy)
                nc.vector.reciprocal_approx_fast(out=rc[0:1, :],
                                                 in_=su[0:1, :])
                nc.gpsimd.partition_broadcast(bc[:], rc[0:1, :])
                for i in range(2):
                    nc.vector.tensor_mul(
                        out=aoT[hp][i * 64:(i + 1) * 64,
                                    qb * 512:(qb + 1) * 512],
                        in0=o_ps[0:64, i * 512:(i + 1) * 512],
                        in1=bc[i * 64:(i + 1) * 64,
                               i * 512:(i + 1) * 512])

            # ---- schedule ----
            def Q(h, b):
                return lambda: q_chunk(h, b)

            def KK(h, b):
                return lambda: k_chunk(h, b)

            def V(s):
                return lambda: v_chunk(s)

            def O(s, b):
                return lambda: out_chunk(s, b)

            # warmup: enough for attention(0, 0) to start: Q0 blk0, K0 all
            # key blocks, V chunks 0..7.
            q_chunk(0, 0)
            for blk in range(NQB):
                k_chunk(0, blk)
            for sc in range(8):
                v_chunk(sc)

            # qb=0 queue: att(0,0) drains V 8..15 at 2/kpp (v12 lands at its
            # kpp2 just in time); att(0,hp) drains its own K/Q pack with
            # k b0 + q b0 at kpp0 so its first energy finds both emitted.
            work += [V(s) for s in range(8, NKC)]
            for hp in range(1, NLC):
                work += [KK(hp, 0), Q(hp, 0), KK(hp, 1), KK(hp, 2), KK(hp, 3)]
            attention(0, 0, pops=(2, 2, 2, 2))
            for hp in range(1, NLC):
                attention(0, hp, pops=(2, 1, 1, 1))
            assert not work

            # qb=1: out chunks for qb0 rows + the q blk1 chunks each
            # att(1,hp) needs at its kpp0, + q blk2 prefetch for qb=2.
            work += [Q(0, 1), O(0, 0), O(0, 1), O(1, 0),
                     Q(1, 1), O(1, 1), O(2, 0), O(2, 1),
                     Q(2, 1), O(3, 0), O(3, 1), Q(0, 2),
                     Q(3, 1), Q(1, 2), Q(2, 2), Q(3, 2)]
            for hp in range(NLC):
                attention(1, hp)
            assert not work

            # qb=2: out chunks for qb1 rows + q blk3 chunks for qb=3.
            work += [O(4, 0), O(4, 1), O(5, 0), Q(0, 3),
                     O(5, 1), O(6, 0), O(6, 1), Q(1, 3),
                     O(7, 0), O(7, 1), Q(2, 3), Q(3, 3)]
            for hp in range(NLC):
                attention(2, hp)
            assert not work

            # qb=3: out chunks for qb2 rows.
            work += [O(s, b) for s in range(8, 12) for b in range(2)]
            for hp in range(NLC):
                attention(3, hp)
            assert not work

            # tail: out chunks for qb3 rows.
            for sc in range(12, NKC):
                for blk in range(2):
                    out_chunk(sc, blk)


def build():
    if "nc" in _CACHE:
        return _CACHE["nc"]
    import concourse.bacc as bacc
    import concourse.mybir as mybir
    import concourse.tile as tile

    f32 = mybir.dt.float32
    bf = mybir.dt.bfloat16
    nc = bacc.Bacc("TRN2", target_bir_lowering=False, debug=False,
                   num_devices=N_CORES)
    xT = nc.dram_tensor("xT", [D, S], bf, kind="ExternalInput").ap()
    modT = nc.dram_tensor("modT", [S, S], bf, kind="ExternalInput").ap()
    wq = nc.dram_tensor("wq", [D, DL], bf, kind="ExternalInput").ap()
    wk = nc.dram_tensor("wk", [D, DL], bf, kind="ExternalInput").ap()
    wv = nc.dram_tensor("wv", [D, DL], bf, kind="ExternalInput").ap()
    wo = nc.dram_tensor("wo", [DL, D], bf, kind="ExternalInput").ap()
    bq = nc.dram_tensor("bq", [DL], f32, kind="ExternalInput").ap()
    bk = nc.dram_tensor("bk", [DL], f32, kind="ExternalInput").ap()
    bo = nc.dram_tensor("bo", [D], bf, kind="ExternalInput").ap()
    out = nc.dram_tensor("out", [S, D], f32, kind="ExternalOutput").ap()

    with tile.TileContext(nc) as tc:
        _emit(nc, tc, mybir, (xT, modT, wq, wk, wv, wo, bq, bk, bo, out))
    nc.compile()
    _CACHE["nc"] = nc
    return nc


def prep_inputs(x, key_padding_mask, attn_mask_modifier, Wq, bq, Wk, bk,
                Wv, bv, Wo, bo):
    """Host-side prep -> per-core in_maps (list of 8 dicts)."""
    x = np.asarray(x, np.float32)
    qmask = np.asarray(key_padding_mask, bool)
    mod = np.asarray(attn_mask_modifier, np.float32)
    Wq = np.asarray(Wq, np.float32); bq = np.asarray(bq, np.float32)
    Wk = np.asarray(Wk, np.float32); bk = np.asarray(bk, np.float32)
    Wv = np.asarray(Wv, np.float32); bv = np.asarray(bv, np.float32)
    Wo = np.asarray(Wo, np.float32); bo = np.asarray(bo, np.float32)

    # fold the query-padding mask into the modifier (masked q row -> energy 0
    # -> uniform softmax, identical to the reference's -1e10 fill)
    modm = mod * (~qmask)[:, :, None].astype(np.float32)   # [b, q, k]

    xT_h, modT_h = [], []
    for b in range(B):
        xT_h.append(np.ascontiguousarray(x[b].T).astype(BF))        # [D, S]
        modT_h.append(np.ascontiguousarray(modm[b].T).astype(BF))   # [k, q]

    half = {}
    for c2 in range(2):
        sl = slice(c2 * DL, (c2 + 1) * DL)
        bo_part = bv[sl] @ Wo[sl] + (bo if c2 == 0 else 0.0)
        half[c2] = {
            "wq": np.ascontiguousarray(Wq[:, sl] * 0.125).astype(BF),
            "wk": np.ascontiguousarray(Wk[:, sl]).astype(BF),
            "wv": np.ascontiguousarray(Wv[:, sl]).astype(BF),
            "wo": np.ascontiguousarray(Wo[sl, :]).astype(BF),
            "bq": (bq[sl] * 0.125).astype(np.float32),
            "bk": bk[sl].astype(np.float32),
            "bo": bo_part.astype(BF),
        }

    in_maps = []
    for c in range(N_CORES):
        b, c2 = divmod(c, 2)
        m = {"xT": xT_h[b], "modT": modT_h[b]}
        m.update(half[c2])
        in_maps.append(m)
    return in_maps


def assemble(results):
    out = np.zeros((B, S, D), np.float32)
    for b in range(B):
        out[b] = results[2 * b]["out"] + results[2 * b + 1]["out"]
    return out


def kernel(**inputs):
    from concourse.bass_utils import run_bass_kernel_spmd
    nc = build()
    in_maps = prep_inputs(**inputs)
    res = run_bass_kernel_spmd(nc, in_maps, list(range(N_CORES)))
    return assemble(res.results)


# revision 16
# speedup vs baseline: 1.0684x; 1.0684x over previous
"""Trainium2 Bass kernel for CustomSelfAttention (B=4, S=2048, D=1024, H=16).

Sharding: 8 cores = batch (4) x head-half (2). Each core projects Q/K/V for
its 8 heads over the full 2048-token sequence, runs attention for those
heads, and computes a PARTIAL output projection (contraction over its 512
model dims). The host sums the two partials per batch during unshard.

Device layout notes:
  - Host pre-transposes x -> x^T [D, S] and mod -> mod^T [S_k, S_q] (bf16).
    Query-padding mask and the 1/sqrt(hd) scale are folded into the mask
    modifier / Wq on the host. bv is folded into the output bias via
    bo' = bv_half @ Wo_half (+ bo on the even core only).
  - Energy is computed transposed (e^T[k, q]) so softmax normalization
    sums arrive via an appended ones-column in the V matmul (M=65), and
    exp() output feeds the attn@V matmul with no transposes.
  - Softmax skips max-subtraction: |energy*mod| <= ~8, exp() is safe.
  - Emission drip-feeds projection/output-chunk work into the attention
    loop so the tensor engine fills the slack while vector (mod-multiply)
    and scalar (exp) run; avoids the long tensor-only warmup.
"""

import numpy as np
import ml_dtypes

B, S, D, H = 4, 2048, 1024, 16
HD = D // H          # 64
HL = H // 2          # 8 local heads per core
DL = HL * HD         # 512 local dims
N_CORES = 8
NDC = D // 128       # 8 dim chunks (full D)
NLC = DL // 128      # 4 local dim chunks
NKC = S // 128       # 16 key chunks
NQB = S // 512       # 4 query blocks
BF = ml_dtypes.bfloat16

_CACHE = {}


def _emit(nc, tc, mybir, io):
    f32 = mybir.dt.float32
    bf = mybir.dt.bfloat16
    Exp = mybir.ActivationFunctionType.Exp
    Copy = mybir.ActivationFunctionType.Copy
    Ident = mybir.ActivationFunctionType.Identity
    mult = mybir.AluOpType.mult
    xT, modT, wq, wk, wv, wo, bqd, bkd, bod, out = io

    from contextlib import ExitStack
    with ExitStack() as _es:
        def _pool(name, bufs, **kw):
            return _es.enter_context(tc.tile_pool(name=name, bufs=bufs, **kw))
        Pv = _pool("pv", NKC)
        Pm = _pool("pmod", 26)
        Pq = _pool("pqT", NLC)
        Pk = _pool("pkT", NLC)
        Pa = _pool("pao", NLC)
        Px = _pool("pxT", NDC)
        Pwq = _pool("pwq", NDC)
        Pwk = _pool("pwk", NDC)
        Pwv = _pool("pwv", NDC)
        Pwo = _pool("pwo", NLC)
        Pe = _pool("pesb", 2)
        Pex = _pool("pex", 2)
        Pbc = _pool("pbc", 1)
        Prs = _pool("prs", 1)
        Po = _pool("pout", 2)
        Pc = _pool("pmisc", 1)

        # ---- constants & weights ----
        bq_sb = Pc.tile([128, NLC], f32, tag="bq")
        bk_sb = Pc.tile([128, NLC], f32, tag="bk")
        nc.sync.dma_start(out=bq_sb[:], in_=bqd[:].rearrange("(c p) -> p c", p=128))
        nc.sync.dma_start(out=bk_sb[:], in_=bkd[:].rearrange("(c p) -> p c", p=128))

        x_sb = []
        for dc in range(NDC):
            t = Px.tile([128, S], bf, tag="xT", name=f"xT{dc}")
            nc.sync.dma_start(out=t[:], in_=xT[dc * 128:(dc + 1) * 128, :])
            x_sb.append(t)
        wq_sb, wk_sb, wv_sb = [], [], []
        for dc in range(NDC):
            t = Pwq.tile([128, DL], bf, tag="wq", name=f"wq{dc}")
            nc.sync.dma_start(out=t[:], in_=wq[dc * 128:(dc + 1) * 128, :])
            wq_sb.append(t)
            t = Pwk.tile([128, DL], bf, tag="wk", name=f"wk{dc}")
            nc.sync.dma_start(out=t[:], in_=wk[dc * 128:(dc + 1) * 128, :])
            wk_sb.append(t)
            t = Pwv.tile([128, DL], bf, tag="wv", name=f"wv{dc}")
            nc.sync.dma_start(out=t[:], in_=wv[dc * 128:(dc + 1) * 128, :])
            wv_sb.append(t)
        wo_sb = []
        for dc in range(NLC):
            t = Pwo.tile([128, D], bf, tag="wo", name=f"wo{dc}")
            nc.sync.dma_start(out=t[:], in_=wo[dc * 128:(dc + 1) * 128, :])
            wo_sb.append(t)

        # mod tiles: [128 keys, 512 queries] per (qb, kc); qb 2,3 reuse
        # qb 0,1 slots (32-buf pool).
        mod_sb = {}
        for qb in range(NQB):
            for kc in range(NKC):
                m = Pm.tile([128, 512], bf, tag="mod", name=f"mod{qb}_{kc}")
                nc.sync.dma_start(
                    out=m[:],
                    in_=modT[kc * 128:(kc + 1) * 128, qb * 512:(qb + 1) * 512])
                mod_sb[(qb, kc)] = m

        v_sb = [Pv.tile([128, HL * 66 + 64], bf, tag="v", name=f"v{i}")
                for i in range(NKC)]
        qT = [Pq.tile([128, S], bf, tag="qT", name=f"qT{i}") for i in range(NLC)]
        kT = [Pk.tile([128, S], bf, tag="kT", name=f"kT{i}") for i in range(NLC)]
        aoT = [Pa.tile([128, S], bf, tag="aoT", name=f"aoT{i}")
               for i in range(NLC)]

        PSB = _pool("psm", 2, space="PSUM")
        PSe = _pool("pse", 2, space="PSUM")
        PSo = _pool("pso", 1, space="PSUM")
        if True:

            def q_chunk(hp, blk):
                # qT[hp][:, blk*512:+512] = (x @ Wq-half)^T chunk + bias
                ps = PSB.tile([128, 512], f32, tag="psb", name=f"psq{hp}_{blk}")
                for kc in range(NDC):
                    nc.tensor.matmul(
                        out=ps[:],
                        lhsT=wq_sb[kc][:, hp * 128:(hp + 1) * 128],
                        rhs=x_sb[kc][:, blk * 512:(blk + 1) * 512],
                        start=(kc == 0), stop=(kc == NDC - 1))
                nc.scalar.activation(
                    out=qT[hp][:, blk * 512:(blk + 1) * 512],
                    in_=ps[:], func=Ident, bias=bq_sb[:, hp:hp + 1])

            def k_chunk(hp, blk):
                ps = PSB.tile([128, 512], f32, tag="psb", name=f"psk{hp}_{blk}")
                for kc in range(NDC):
                    nc.tensor.matmul(
                        out=ps[:],
                        lhsT=wk_sb[kc][:, hp * 128:(hp + 1) * 128],
                        rhs=x_sb[kc][:, blk * 512:(blk + 1) * 512],
                        start=(kc == 0), stop=(kc == NDC - 1))
                nc.scalar.activation(
                    out=kT[hp][:, blk * 512:(blk + 1) * 512],
                    in_=ps[:], func=Ident, bias=bk_sb[:, hp:hp + 1])

            def v_chunk(sc):
                # v rows for seq chunk sc, all 8 local heads (+ ones col)
                ps = PSB.tile([128, 512], f32, tag="psb", name=f"psv{sc}")
                for dc in range(NDC):
                    nc.tensor.matmul(
                        out=ps[:],
                        lhsT=x_sb[dc][:, sc * 128:(sc + 1) * 128],
                        rhs=wv_sb[dc][:],
                        start=(dc == 0), stop=(dc == NDC - 1))
                v3 = v_sb[sc][:, 0:HL * 66].rearrange("p (h d) -> p h d", d=66)
                nc.gpsimd.memset(v3[:, :, 64:65], 1.0)
                nc.gpsimd.memset(v_sb[sc][:, HL * 66:], 0.0)
                nc.scalar.activation(
                    out=v3[:, :, 0:64],
                    in_=ps[:].rearrange("p (h d) -> p h d", d=64),
                    func=Copy)

            def out_chunk(sc, blk):
                # partial out rows sc*128..+128, cols blk*512..+512
                pf = PSB.tile([128, 512], f32, tag="psb", name=f"pf{sc}_{blk}")
                for dc in range(NLC):
                    nc.tensor.matmul(
                        out=pf[:],
                        lhsT=aoT[dc][:, sc * 128:(sc + 1) * 128],
                        rhs=wo_sb[dc][:, blk * 512:(blk + 1) * 512],
                        start=(dc == 0), stop=(dc == NLC - 1))
                osb = Po.tile([128, 512], f32, tag="osb", name=f"osb{sc}_{blk}")
                nc.scalar.activation(out=osb[:], in_=pf[:], func=Copy)
                nc.sync.dma_start(
                    out=out[sc * 128:(sc + 1) * 128,
                            blk * 512:(blk + 1) * 512],
                    in_=osb[:])

            # drip-feed queue: thunks of ~2us tensor work popped inside the
            # attention loop at a controlled per-kpp rate. Order matters: a
            # thunk must be emitted no later than the kpp whose energy/attnV
            # matmuls consume its output (engine queues run in emission
            # order, so a consumer emitted before its producer deadlocks).
            work = []

            def attention(qb, hp, pops=(1, 1, 1, 1)):
                o_ps = [PSo.tile([128, 512], f32, tag=f"o{i}",
                                 name=f"o{i}_{qb}_{hp}") for i in (0, 1)]
                for kpp in range(NKC // 4):
                    for _ in range(pops[kpp]):
                        if work:
                            work.pop(0)()
                    e4 = Pe.tile([128, 4096], bf, tag="e",
                                 name=f"e{qb}_{hp}_{kpp}")
                    ex4 = Pex.tile([128, 4096], bf, tag="ex",
                                   name=f"ex{qb}_{hp}_{kpp}")
                    for jj in range(4):
                        kc = kpp * 4 + jj
                        pe_ps = PSe.tile([128, 1024], f32, tag="pe",
                                         name=f"pe{qb}_{hp}_{kc}")
                        for i in range(2):
                            nc.tensor.matmul(
                                out=pe_ps[:, i * 512:(i + 1) * 512],
                                lhsT=kT[hp][i * 64:(i + 1) * 64,
                                            kc * 128:(kc + 1) * 128],
                                rhs=qT[hp][i * 64:(i + 1) * 64,
                                           qb * 512:(qb + 1) * 512],
                                start=True, stop=True)
                        rep = (mod_sb[(qb, kc)][:, 0:512]
                               .unsqueeze(1).broadcast_to((128, 2, 512)))
                        nc.vector.scalar_tensor_tensor(
                            out=e4[:, jj * 1024:(jj + 1) * 1024]
                                .rearrange("p (r c) -> p r c", r=2),
                            in0=pe_ps[:].rearrange("p (r c) -> p r c", r=2),
                            scalar=1.0, in1=rep, op0=mult, op1=mult)
                    nc.scalar.activation(out=ex4[:], in_=e4[:], func=Exp)
                    for jj in range(4):
                        kc = kpp * 4 + jj
                        for i in range(2):
                            h = hp * 2 + i
                            nc.tensor.matmul(
                                out=o_ps[i][:],
                                lhsT=v_sb[kc][:, h * 66:h * 66 + 128],
                                rhs=ex4[:, jj * 1024 + i * 512:
                                        jj * 1024 + (i + 1) * 512],
                                start=(kc == 0), stop=(kc == NKC - 1))
                # normalize: sums live in psum row 64 (ones column). recip
                # reads psum directly; partition_broadcast (GpSimd, ~1us)
                # hides behind the other head's work.
                bcs = []
                for i in range(2):
                    su = Prs.tile([1, 512], f32, tag="su",
                                  name=f"su{qb}_{hp}_{i}")
                    rc = Prs.tile([1, 512], f32, tag="rc",
                                  name=f"rc{qb}_{hp}_{i}")
                    bc = Pbc.tile([128, 512], f32, tag=f"bc{i}",
                                  name=f"bc{qb}_{hp}_{i}")
                    nc.scalar.activation(out=su[0:1, :],
                                         in_=o_ps[i][64:65, :], func=Copy)
                    nc.vector.reciprocal_approx_fast(out=rc[0:1, :],
                                                     in_=su[0:1, :])
                    nc.gpsimd.partition_broadcast(bc[:], rc[0:1, :])
                    bcs.append(bc)
                for i in range(2):
                    nc.vector.tensor_mul(
                        out=aoT[hp][i * 64:(i + 1) * 64,
                                    qb * 512:(qb + 1) * 512],
                        in0=o_ps[i][0:64, :],
                        in1=bcs[i][i * 64:(i + 1) * 64, :])

            # ---- schedule ----
            def Q(h, b):
                return lambda: q_chunk(h, b)

            def KK(h, b):
                return lambda: k_chunk(h, b)

            def V(s):
                return lambda: v_chunk(s)

            def O(s, b):
                return lambda: out_chunk(s, b)

            # warmup: enough for attention(0, 0) to start: Q0 blk0, K0 all
            # key blocks, V chunks 0..7.
            q_chunk(0, 0)
            for blk in range(NQB):
                k_chunk(0, blk)
            for sc in range(8):
                v_chunk(sc)

            # qb=0 queue: att(0,0) drains V 8..15 at 2/kpp (v12 lands at its
            # kpp2 just in time); att(0,hp) drains its own K/Q pack with
            # k b0 + q b0 at kpp0 so its first energy finds both emitted.
            work += [V(s) for s in range(8, NKC)]
            for hp in range(1, NLC):
                work += [KK(hp, 0), Q(hp, 0), KK(hp, 1), KK(hp, 2), KK(hp, 3)]
            attention(0, 0, pops=(2, 2, 2, 2))
            for hp in range(1, NLC):
                attention(0, hp, pops=(2, 1, 1, 1))
            assert not work

            # qb=1: out chunks for qb0 rows + the q blk1 chunks each
            # att(1,hp) needs at its kpp0, + q blk2 prefetch for qb=2.
            work += [Q(0, 1), O(0, 0), O(0, 1), O(1, 0),
                     Q(1, 1), O(1, 1), O(2, 0), O(2, 1),
                     Q(2, 1), O(3, 0), O(3, 1), Q(0, 2),
                     Q(3, 1), Q(1, 2), Q(2, 2), Q(3, 2)]
            for hp in range(NLC):
                attention(1, hp)
            assert not work

            # qb=2: out chunks for qb1 rows + q blk3 chunks for qb=3.
            work += [O(4, 0), O(4, 1), O(5, 0), Q(0, 3),
                     O(5, 1), O(6, 0), O(6, 1), Q(1, 3),
                     O(7, 0), O(7, 1), Q(2, 3), Q(3, 3)]
            for hp in range(NLC):
                attention(2, hp)
            assert not work

            # qb=3: out chunks for qb2 rows.
            work += [O(s, b) for s in range(8, 12) for b in range(2)]
            for hp in range(NLC):
                attention(3, hp)
            assert not work

            # tail: out chunks for qb3 rows.
            for sc in range(12, NKC):
                for blk in range(2):
                    out_chunk(sc, blk)


def build():
    if "nc" in _CACHE:
        return _CACHE["nc"]
    import concourse.bacc as bacc
    import concourse.mybir as mybir
    import concourse.tile as tile

    f32 = mybir.dt.float32
    bf = mybir.dt.bfloat16
    nc = bacc.Bacc("TRN2", target_bir_lowering=False, debug=False,
                   num_devices=N_CORES)
    xT = nc.dram_tensor("xT", [D, S], bf, kind="ExternalInput").ap()
    modT = nc.dram_tensor("modT", [S, S], bf, kind="ExternalInput").ap()
    wq = nc.dram_tensor("wq", [D, DL], bf, kind="ExternalInput").ap()
    wk = nc.dram_tensor("wk", [D, DL], bf, kind="ExternalInput").ap()
    wv = nc.dram_tensor("wv", [D, DL], bf, kind="ExternalInput").ap()
    wo = nc.dram_tensor("wo", [DL, D], bf, kind="ExternalInput").ap()
    bq = nc.dram_tensor("bq", [DL], f32, kind="ExternalInput").ap()
    bk = nc.dram_tensor("bk", [DL], f32, kind="ExternalInput").ap()
    bo = nc.dram_tensor("bo", [D], bf, kind="ExternalInput").ap()
    out = nc.dram_tensor("out", [S, D], f32, kind="ExternalOutput").ap()

    with tile.TileContext(nc) as tc:
        _emit(nc, tc, mybir, (xT, modT, wq, wk, wv, wo, bq, bk, bo, out))
    nc.compile()
    _CACHE["nc"] = nc
    return nc


def prep_inputs(x, key_padding_mask, attn_mask_modifier, Wq, bq, Wk, bk,
                Wv, bv, Wo, bo):
    """Host-side prep -> per-core in_maps (list of 8 dicts)."""
    x = np.asarray(x, np.float32)
    qmask = np.asarray(key_padding_mask, bool)
    mod = np.asarray(attn_mask_modifier, np.float32)
    Wq = np.asarray(Wq, np.float32); bq = np.asarray(bq, np.float32)
    Wk = np.asarray(Wk, np.float32); bk = np.asarray(bk, np.float32)
    Wv = np.asarray(Wv, np.float32); bv = np.asarray(bv, np.float32)
    Wo = np.asarray(Wo, np.float32); bo = np.asarray(bo, np.float32)

    # fold the query-padding mask into the modifier (masked q row -> energy 0
    # -> uniform softmax, identical to the reference's -1e10 fill)
    modm = mod * (~qmask)[:, :, None].astype(np.float32)   # [b, q, k]

    xT_h, modT_h = [], []
    for b in range(B):
        xT_h.append(np.ascontiguousarray(x[b].T).astype(BF))        # [D, S]
        modT_h.append(np.ascontiguousarray(modm[b].T).astype(BF))   # [k, q]

    half = {}
    for c2 in range(2):
        sl = slice(c2 * DL, (c2 + 1) * DL)
        bo_part = np.zeros_like(bo)
        half[c2] = {
            "wq": np.ascontiguousarray(Wq[:, sl] * 0.125).astype(BF),
            "wk": np.ascontiguousarray(Wk[:, sl]).astype(BF),
            "wv": np.ascontiguousarray(Wv[:, sl]).astype(BF),
            "wo": np.ascontiguousarray(Wo[sl, :]).astype(BF),
            "bq": (bq[sl] * 0.125).astype(np.float32),
            "bk": bk[sl].astype(np.float32),
            "bo": bo_part.astype(BF),
        }

    in_maps = []
    for c in range(N_CORES):
        b, c2 = divmod(c, 2)
        m = {"xT": xT_h[b], "modT": modT_h[b]}
        m.update(half[c2])
        in_maps.append(m)
    return in_maps


def assemble(results, bias):
    out = np.empty((B, S, D), np.float32)
    for b in range(B):
        np.add(results[2 * b]["out"], results[2 * b + 1]["out"], out=out[b])
        out[b] += bias
    return out


def full_bias(Wo, bv, bo):
    return (np.asarray(bv, np.float32) @ np.asarray(Wo, np.float32)
            + np.asarray(bo, np.float32))


def kernel(**inputs):
    from concourse.bass_utils import run_bass_kernel_spmd
    nc = build()
    in_maps = prep_inputs(**inputs)
    res = run_bass_kernel_spmd(nc, in_maps, list(range(N_CORES)))
    return assemble(res.results, full_bias(inputs["Wo"], inputs["bv"],
                                           inputs["bo"]))


# revision 18
# speedup vs baseline: 1.0856x; 1.0161x over previous
"""Trainium2 Bass kernel for CustomSelfAttention (B=4, S=2048, D=1024, H=16).

Sharding: 8 cores = batch (4) x head-half (2). Each core projects Q/K/V for
its 8 heads over the full 2048-token sequence, runs attention for those
heads, and computes a PARTIAL output projection (contraction over its 512
model dims). The host sums the two partials per batch during unshard.

Device layout notes:
  - Host pre-transposes x -> x^T [D, S] and mod -> mod^T [S_k, S_q] (bf16).
    Query-padding mask and the 1/sqrt(hd) scale are folded into the mask
    modifier / Wq on the host. bv is folded into the output bias via
    bo' = bv_half @ Wo_half (+ bo on the even core only).
  - Energy is computed transposed (e^T[k, q]) so softmax normalization
    sums arrive via an appended ones-column in the V matmul (M=65), and
    exp() output feeds the attn@V matmul with no transposes.
  - Softmax skips max-subtraction: |energy*mod| <= ~8, exp() is safe.
  - Emission drip-feeds projection/output-chunk work into the attention
    loop so the tensor engine fills the slack while vector (mod-multiply)
    and scalar (exp) run; avoids the long tensor-only warmup.
"""

import numpy as np
import ml_dtypes

B, S, D, H = 4, 2048, 1024, 16
HD = D // H          # 64
HL = H // 2          # 8 local heads per core
DL = HL * HD         # 512 local dims
N_CORES = 8
NDC = D // 128       # 8 dim chunks (full D)
NLC = DL // 128      # 4 local dim chunks
NKC = S // 128       # 16 key chunks
NQB = S // 512       # 4 query blocks
BF = ml_dtypes.bfloat16

_CACHE = {}


def _emit(nc, tc, mybir, io):
    f32 = mybir.dt.float32
    bf = mybir.dt.bfloat16
    Exp = mybir.ActivationFunctionType.Exp
    Copy = mybir.ActivationFunctionType.Copy
    Ident = mybir.ActivationFunctionType.Identity
    mult = mybir.AluOpType.mult
    xT, modT, wq, wk, wv, wo, bqd, bkd, bod, out = io

    from contextlib import ExitStack
    with ExitStack() as _es:
        def _pool(name, bufs, **kw):
            return _es.enter_context(tc.tile_pool(name=name, bufs=bufs, **kw))
        Pv = _pool("pv", NKC)
        Pm = _pool("pmod", 26)
        Pq = _pool("pqT", NLC)
        Pk = _pool("pkT", NLC)
        Pa = _pool("pao", NLC)
        Px = _pool("pxT", NDC)
        Pwq = _pool("pwq", NDC)
        Pwk = _pool("pwk", NDC)
        Pwv = _pool("pwv", NDC)
        Pwo = _pool("pwo", NLC)
        Pe = _pool("pesb", 2)
        Pex = _pool("pex", 2)
        Pbc = _pool("pbc", 1)
        Prs = _pool("prs", 1)
        Po = _pool("pout", 2)
        Pc = _pool("pmisc", 1)

        # ---- constants & weights ----
        bq_sb = Pc.tile([128, NLC], f32, tag="bq")
        bk_sb = Pc.tile([128, NLC], f32, tag="bk")
        nc.sync.dma_start(out=bq_sb[:], in_=bqd[:].rearrange("(c p) -> p c", p=128))
        nc.sync.dma_start(out=bk_sb[:], in_=bkd[:].rearrange("(c p) -> p c", p=128))

        x_sb = []
        for dc in range(NDC):
            t = Px.tile([128, S], bf, tag="xT", name=f"xT{dc}")
            nc.sync.dma_start(out=t[:], in_=xT[dc * 128:(dc + 1) * 128, :])
            x_sb.append(t)
        wq_sb, wk_sb, wv_sb = [], [], []
        for dc in range(NDC):
            t = Pwq.tile([128, DL], bf, tag="wq", name=f"wq{dc}")
            nc.sync.dma_start(out=t[:], in_=wq[dc * 128:(dc + 1) * 128, :])
            wq_sb.append(t)
            t = Pwk.tile([128, DL], bf, tag="wk", name=f"wk{dc}")
            nc.sync.dma_start(out=t[:], in_=wk[dc * 128:(dc + 1) * 128, :])
            wk_sb.append(t)
            t = Pwv.tile([128, DL], bf, tag="wv", name=f"wv{dc}")
            nc.sync.dma_start(out=t[:], in_=wv[dc * 128:(dc + 1) * 128, :])
            wv_sb.append(t)
        wo_sb = []
        for dc in range(NLC):
            t = Pwo.tile([128, D], bf, tag="wo", name=f"wo{dc}")
            nc.sync.dma_start(out=t[:], in_=wo[dc * 128:(dc + 1) * 128, :])
            wo_sb.append(t)

        # mod tiles: [128 keys, 512 queries] per (qb, kc); qb 2,3 reuse
        # qb 0,1 slots (32-buf pool).
        mod_sb = {}
        for qb in range(NQB):
            for kc in range(NKC):
                m = Pm.tile([128, 512], bf, tag="mod", name=f"mod{qb}_{kc}")
                nc.sync.dma_start(
                    out=m[:],
                    in_=modT[kc * 128:(kc + 1) * 128, qb * 512:(qb + 1) * 512])
                mod_sb[(qb, kc)] = m

        v_sb = [Pv.tile([128, HL * 66 + 64], bf, tag="v", name=f"v{i}")
                for i in range(NKC)]
        qT = [Pq.tile([128, S], bf, tag="qT", name=f"qT{i}") for i in range(NLC)]
        kT = [Pk.tile([128, S], bf, tag="kT", name=f"kT{i}") for i in range(NLC)]
        aoT = [Pa.tile([128, S], bf, tag="aoT", name=f"aoT{i}")
               for i in range(NLC)]

        PSB = _pool("psm", 2, space="PSUM")
        PSe = _pool("pse", 2, space="PSUM")
        PSo = _pool("pso", 1, space="PSUM")
        if True:

            def q_chunk(hp, blk):
                # qT[hp][:, blk*512:+512] = (x @ Wq-half)^T chunk + bias
                ps = PSB.tile([128, 512], f32, tag="psb", name=f"psq{hp}_{blk}")
                for kc in range(NDC):
                    nc.tensor.matmul(
                        out=ps[:],
                        lhsT=wq_sb[kc][:, hp * 128:(hp + 1) * 128],
                        rhs=x_sb[kc][:, blk * 512:(blk + 1) * 512],
                        start=(kc == 0), stop=(kc == NDC - 1))
                nc.scalar.activation(
                    out=qT[hp][:, blk * 512:(blk + 1) * 512],
                    in_=ps[:], func=Ident, bias=bq_sb[:, hp:hp + 1])

            def k_chunk(hp, blk):
                ps = PSB.tile([128, 512], f32, tag="psb", name=f"psk{hp}_{blk}")
                for kc in range(NDC):
                    nc.tensor.matmul(
                        out=ps[:],
                        lhsT=wk_sb[kc][:, hp * 128:(hp + 1) * 128],
                        rhs=x_sb[kc][:, blk * 512:(blk + 1) * 512],
                        start=(kc == 0), stop=(kc == NDC - 1))
                nc.scalar.activation(
                    out=kT[hp][:, blk * 512:(blk + 1) * 512],
                    in_=ps[:], func=Ident, bias=bk_sb[:, hp:hp + 1])

            def v_chunk(sc):
                # v rows for seq chunk sc, all 8 local heads (+ ones col)
                ps = PSB.tile([128, 512], f32, tag="psb", name=f"psv{sc}")
                for dc in range(NDC):
                    nc.tensor.matmul(
                        out=ps[:],
                        lhsT=x_sb[dc][:, sc * 128:(sc + 1) * 128],
                        rhs=wv_sb[dc][:],
                        start=(dc == 0), stop=(dc == NDC - 1))
                v3 = v_sb[sc][:, 0:HL * 66].rearrange("p (h d) -> p h d", d=66)
                nc.gpsimd.memset(v3[:, :, 64:65], 1.0)
                nc.gpsimd.memset(v_sb[sc][:, HL * 66:], 0.0)
                nc.scalar.activation(
                    out=v3[:, :, 0:64],
                    in_=ps[:].rearrange("p (h d) -> p h d", d=64),
                    func=Copy)

            def out_chunk(sc, blk):
                # partial out rows sc*128..+128, cols blk*512..+512
                pf = PSB.tile([128, 512], f32, tag="psb", name=f"pf{sc}_{blk}")
                for dc in range(NLC):
                    nc.tensor.matmul(
                        out=pf[:],
                        lhsT=aoT[dc][:, sc * 128:(sc + 1) * 128],
                        rhs=wo_sb[dc][:, blk * 512:(blk + 1) * 512],
                        start=(dc == 0), stop=(dc == NLC - 1))
                osb = Po.tile([128, 512], f32, tag="osb", name=f"osb{sc}_{blk}")
                nc.scalar.activation(out=osb[:], in_=pf[:], func=Copy)
                nc.sync.dma_start(
                    out=out[sc * 128:(sc + 1) * 128,
                            blk * 512:(blk + 1) * 512],
                    in_=osb[:])

            # drip-feed queue: thunks of ~2us tensor work popped inside the
            # attention loop at a controlled per-kpp rate. Order matters: a
            # thunk must be emitted no later than the kpp whose energy/attnV
            # matmuls consume its output (engine queues run in emission
            # order, so a consumer emitted before its producer deadlocks).
            work = []

            def attention(qb, hp, pre=(), mid=()):
                # pre: thunks the call's first energy depends on (Q/K chunks)
                # mid: safe filler popped at the stt-wait point of each kpp.
                # attnV runs one kpp behind exp so it never waits on it.
                for t in pre:
                    t()
                mid = list(mid)
                rate = -(-len(mid) // 4) if mid else 0
                o_ps = [PSo.tile([128, 512], f32, tag=f"o{i}",
                                 name=f"o{i}_{qb}_{hp}") for i in (0, 1)]
                ex_q = []

                def attnv(kpp, ex4):
                    for jj in range(4):
                        kc = kpp * 4 + jj
                        for i in range(2):
                            h = hp * 2 + i
                            nc.tensor.matmul(
                                out=o_ps[i][:],
                                lhsT=v_sb[kc][:, h * 66:h * 66 + 128],
                                rhs=ex4[:, jj * 1024 + i * 512:
                                        jj * 1024 + (i + 1) * 512],
                                start=(kc == 0), stop=(kc == NKC - 1))

                for kpp in range(NKC // 4):
                    e4 = Pe.tile([128, 4096], bf, tag="e",
                                 name=f"e{qb}_{hp}_{kpp}")
                    ex4 = Pex.tile([128, 4096], bf, tag="ex",
                                   name=f"ex{qb}_{hp}_{kpp}")
                    for jj in range(4):
                        kc = kpp * 4 + jj
                        pe_ps = PSe.tile([128, 1024], f32, tag="pe",
                                         name=f"pe{qb}_{hp}_{kc}")
                        for i in range(2):
                            nc.tensor.matmul(
                                out=pe_ps[:, i * 512:(i + 1) * 512],
                                lhsT=kT[hp][i * 64:(i + 1) * 64,
                                            kc * 128:(kc + 1) * 128],
                                rhs=qT[hp][i * 64:(i + 1) * 64,
                                           qb * 512:(qb + 1) * 512],
                                start=True, stop=True)
                        rep = (mod_sb[(qb, kc)][:, 0:512]
                               .unsqueeze(1).broadcast_to((128, 2, 512)))
                        nc.vector.scalar_tensor_tensor(
                            out=e4[:, jj * 1024:(jj + 1) * 1024]
                                .rearrange("p (r c) -> p r c", r=2),
                            in0=pe_ps[:].rearrange("p (r c) -> p r c", r=2),
                            scalar=1.0, in1=rep, op0=mult, op1=mult)
                        if jj == 1:
                            for _ in range(rate):
                                if mid:
                                    mid.pop(0)()
                    nc.scalar.activation(out=ex4[:], in_=e4[:], func=Exp)
                    ex_q.append((kpp, ex4))
                    if kpp > 0:
                        attnv(*ex_q.pop(0))
                attnv(*ex_q.pop(0))
                while mid:
                    mid.pop(0)()
                # normalize: sums live in psum row 64 (ones column). recip
                # reads psum directly; partition_broadcast (GpSimd, ~1us)
                # hides behind the other head's work.
                bcs = []
                for i in range(2):
                    su = Prs.tile([1, 512], f32, tag="su",
                                  name=f"su{qb}_{hp}_{i}")
                    rc = Prs.tile([1, 512], f32, tag="rc",
                                  name=f"rc{qb}_{hp}_{i}")
                    bc = Pbc.tile([128, 512], f32, tag=f"bc{i}",
                                  name=f"bc{qb}_{hp}_{i}")
                    nc.scalar.activation(out=su[0:1, :],
                                         in_=o_ps[i][64:65, :], func=Copy)
                    nc.vector.reciprocal_approx_fast(out=rc[0:1, :],
                                                     in_=su[0:1, :])
                    nc.gpsimd.partition_broadcast(bc[:], rc[0:1, :])
                    bcs.append(bc)
                for i in range(2):
                    nc.vector.tensor_mul(
                        out=aoT[hp][i * 64:(i + 1) * 64,
                                    qb * 512:(qb + 1) * 512],
                        in0=o_ps[i][0:64, :],
                        in1=bcs[i][i * 64:(i + 1) * 64, :])

            # ---- schedule ----
            def Q(h, b):
                return lambda: q_chunk(h, b)

            def KK(h, b):
                return lambda: k_chunk(h, b)

            def V(s):
                return lambda: v_chunk(s)

            def O(s, b):
                return lambda: out_chunk(s, b)

            # warmup: enough for attention(0, 0) to start: Q0 blk0, K0 all
            # key blocks, V chunks 0..7.
            q_chunk(0, 0)
            for blk in range(NQB):
                k_chunk(0, blk)
            for sc in range(8):
                v_chunk(sc)

            # qb=0: att(0,0) drains V 8..15 as mid filler; att(0,hp) needs
            # its K blk0 + Q blk0 as pre, rest of K as mid.
            attention(0, 0, mid=[V(s) for s in range(8, NKC)])
            for hp in range(1, NLC):
                attention(0, hp, pre=[KK(hp, 0), Q(hp, 0)],
                          mid=[KK(hp, 1), KK(hp, 2), KK(hp, 3)])

            # qb>=1: each call pre-emits the Q block it reads; out chunks for
            # the previous qb's rows drip through the mid slots.
            for qb in range(1, NQB):
                oc = [O(s, b) for s in range((qb - 1) * 4, qb * 4)
                      for b in range(2)]
                for hp in range(NLC):
                    attention(qb, hp, pre=[Q(hp, qb)],
                              mid=oc[hp * 2:hp * 2 + 2])

            # tail: out chunks for qb3 rows.
            for sc in range((NQB - 1) * 4, NQB * 4):
                for blk in range(2):
                    out_chunk(sc, blk)


def build():
    if "nc" in _CACHE:
        return _CACHE["nc"]
    import concourse.bacc as bacc
    import concourse.mybir as mybir
    import concourse.tile as tile

    f32 = mybir.dt.float32
    bf = mybir.dt.bfloat16
    nc = bacc.Bacc("TRN2", target_bir_lowering=False, debug=False,
                   num_devices=N_CORES)
    xT = nc.dram_tensor("xT", [D, S], bf, kind="ExternalInput").ap()
    modT = nc.dram_tensor("modT", [S, S], bf, kind="ExternalInput").ap()
    wq = nc.dram_tensor("wq", [D, DL], bf, kind="ExternalInput").ap()
    wk = nc.dram_tensor("wk", [D, DL], bf, kind="ExternalInput").ap()
    wv = nc.dram_tensor("wv", [D, DL], bf, kind="ExternalInput").ap()
    wo = nc.dram_tensor("wo", [DL, D], bf, kind="ExternalInput").ap()
    bq = nc.dram_tensor("bq", [DL], f32, kind="ExternalInput").ap()
    bk = nc.dram_tensor("bk", [DL], f32, kind="ExternalInput").ap()
    bo = nc.dram_tensor("bo", [D], bf, kind="ExternalInput").ap()
    out = nc.dram_tensor("out", [S, D], f32, kind="ExternalOutput").ap()

    with tile.TileContext(nc) as tc:
        _emit(nc, tc, mybir, (xT, modT, wq, wk, wv, wo, bq, bk, bo, out))
    nc.compile()
    _CACHE["nc"] = nc
    return nc


def prep_inputs(x, key_padding_mask, attn_mask_modifier, Wq, bq, Wk, bk,
                Wv, bv, Wo, bo):
    """Host-side prep -> per-core in_maps (list of 8 dicts)."""
    x = np.asarray(x, np.float32)
    qmask = np.asarray(key_padding_mask, bool)
    mod = np.asarray(attn_mask_modifier, np.float32)
    Wq = np.asarray(Wq, np.float32); bq = np.asarray(bq, np.float32)
    Wk = np.asarray(Wk, np.float32); bk = np.asarray(bk, np.float32)
    Wv = np.asarray(Wv, np.float32); bv = np.asarray(bv, np.float32)
    Wo = np.asarray(Wo, np.float32); bo = np.asarray(bo, np.float32)

    # fold the query-padding mask into the modifier (masked q row -> energy 0
    # -> uniform softmax, identical to the reference's -1e10 fill)
    modm = mod * (~qmask)[:, :, None].astype(np.float32)   # [b, q, k]

    xT_h, modT_h = [], []
    for b in range(B):
        xT_h.append(np.ascontiguousarray(x[b].T).astype(BF))        # [D, S]
        modT_h.append(np.ascontiguousarray(modm[b].T).astype(BF))   # [k, q]

    half = {}
    for c2 in range(2):
        sl = slice(c2 * DL, (c2 + 1) * DL)
        bo_part = np.zeros_like(bo)
        half[c2] = {
            "wq": np.ascontiguousarray(Wq[:, sl] * 0.125).astype(BF),
            "wk": np.ascontiguousarray(Wk[:, sl]).astype(BF),
            "wv": np.ascontiguousarray(Wv[:, sl]).astype(BF),
            "wo": np.ascontiguousarray(Wo[sl, :]).astype(BF),
            "bq": (bq[sl] * 0.125).astype(np.float32),
            "bk": bk[sl].astype(np.float32),
            "bo": bo_part.astype(BF),
        }

    in_maps = []
    for c in range(N_CORES):
        b, c2 = divmod(c, 2)
        m = {"xT": xT_h[b], "modT": modT_h[b]}
        m.update(half[c2])
        in_maps.append(m)
    return in_maps


def assemble(results, bias):
    out = np.empty((B, S, D), np.float32)
    for b in range(B):
        np.add(results[2 * b]["out"], results[2 * b + 1]["out"], out=out[b])
        out[b] += bias
    return out


def full_bias(Wo, bv, bo):
    return (np.asarray(bv, np.float32) @ np.asarray(Wo, np.float32)
            + np.asarray(bo, np.float32))


def kernel(**inputs):
    from concourse.bass_utils import run_bass_kernel_spmd
    nc = build()
    in_maps = prep_inputs(**inputs)
    res = run_bass_kernel_spmd(nc, in_maps, list(range(N_CORES)))
    return assemble(res.results, full_bias(inputs["Wo"], inputs["bv"],
                                           inputs["bo"]))


# revision 19
# speedup vs baseline: 1.1094x; 1.0219x over previous
"""Trainium2 Bass kernel for CustomSelfAttention (B=4, S=2048, D=1024, H=16).

Sharding: 8 cores = batch (4) x head-half (2). Each core projects Q/K/V for
its 8 heads over the full 2048-token sequence, runs attention for those
heads, and computes a PARTIAL output projection (contraction over its 512
model dims). The host sums the two partials per batch during unshard.

Device layout notes:
  - Host pre-transposes x -> x^T [D, S] and mod -> mod^T [S_k, S_q] (bf16).
    Query-padding mask and the 1/sqrt(hd) scale are folded into the mask
    modifier / Wq on the host. bv is folded into the output bias via
    bo' = bv_half @ Wo_half (+ bo on the even core only).
  - Energy is computed transposed (e^T[k, q]) so softmax normalization
    sums arrive via an appended ones-column in the V matmul (M=65), and
    exp() output feeds the attn@V matmul with no transposes.
  - Softmax skips max-subtraction: |energy*mod| <= ~8, exp() is safe.
  - Emission drip-feeds projection/output-chunk work into the attention
    loop so the tensor engine fills the slack while vector (mod-multiply)
    and scalar (exp) run; avoids the long tensor-only warmup.
"""

import numpy as np
import ml_dtypes

B, S, D, H = 4, 2048, 1024, 16
HD = D // H          # 64
HL = H // 2          # 8 local heads per core
DL = HL * HD         # 512 local dims
N_CORES = 8
NDC = D // 128       # 8 dim chunks (full D)
NLC = DL // 128      # 4 local dim chunks
NKC = S // 128       # 16 key chunks
NQB = S // 512       # 4 query blocks
BF = ml_dtypes.bfloat16

_CACHE = {}


def _emit(nc, tc, mybir, io):
    f32 = mybir.dt.float32
    bf = mybir.dt.bfloat16
    Exp = mybir.ActivationFunctionType.Exp
    Copy = mybir.ActivationFunctionType.Copy
    Ident = mybir.ActivationFunctionType.Identity
    mult = mybir.AluOpType.mult
    xT, modT, wq, wk, wv, wo, bqd, bkd, bod, out = io

    from contextlib import ExitStack
    with ExitStack() as _es:
        def _pool(name, bufs, **kw):
            return _es.enter_context(tc.tile_pool(name=name, bufs=bufs, **kw))
        Pv = _pool("pv", NKC)
        Pm = _pool("pmod", 26)
        Pq = _pool("pqT", NLC)
        Pk = _pool("pkT", NLC)
        Pa = _pool("pao", NLC)
        Px = _pool("pxT", NDC)
        Pwq = _pool("pwq", NDC)
        Pwk = _pool("pwk", NDC)
        Pwv = _pool("pwv", NDC)
        Pwo = _pool("pwo", NLC)
        Pe = _pool("pesb", 2)
        Pex = _pool("pex", 2)
        Pbc = _pool("pbc", 1)
        Prs = _pool("prs", 1)
        Po = _pool("pout", 2)
        Pc = _pool("pmisc", 1)

        # ---- constants & weights ----
        bq_sb = Pc.tile([128, NLC], f32, tag="bq")
        bk_sb = Pc.tile([128, NLC], f32, tag="bk")
        nc.sync.dma_start(out=bq_sb[:], in_=bqd[:].rearrange("(c p) -> p c", p=128))
        nc.sync.dma_start(out=bk_sb[:], in_=bkd[:].rearrange("(c p) -> p c", p=128))

        x_sb = []
        for dc in range(NDC):
            t = Px.tile([128, S], bf, tag="xT", name=f"xT{dc}")
            nc.sync.dma_start(out=t[:], in_=xT[dc * 128:(dc + 1) * 128, :])
            x_sb.append(t)
        wq_sb, wk_sb, wv_sb = [], [], []
        for dc in range(NDC):
            t = Pwq.tile([128, DL], bf, tag="wq", name=f"wq{dc}")
            nc.sync.dma_start(out=t[:], in_=wq[dc * 128:(dc + 1) * 128, :])
            wq_sb.append(t)
            t = Pwk.tile([128, DL], bf, tag="wk", name=f"wk{dc}")
            nc.sync.dma_start(out=t[:], in_=wk[dc * 128:(dc + 1) * 128, :])
            wk_sb.append(t)
            t = Pwv.tile([128, DL], bf, tag="wv", name=f"wv{dc}")
            nc.sync.dma_start(out=t[:], in_=wv[dc * 128:(dc + 1) * 128, :])
            wv_sb.append(t)
        wo_sb = []
        for dc in range(NLC):
            t = Pwo.tile([128, D], bf, tag="wo", name=f"wo{dc}")
            nc.sync.dma_start(out=t[:], in_=wo[dc * 128:(dc + 1) * 128, :])
            wo_sb.append(t)

        # mod tiles: [128 keys, 512 queries] per (qb, kc); qb 2,3 reuse
        # qb 0,1 slots (32-buf pool).
        mod_sb = {}
        for qb in range(NQB):
            for kc in range(NKC):
                m = Pm.tile([128, 512], bf, tag="mod", name=f"mod{qb}_{kc}")
                nc.sync.dma_start(
                    out=m[:],
                    in_=modT[kc * 128:(kc + 1) * 128, qb * 512:(qb + 1) * 512])
                mod_sb[(qb, kc)] = m

        v_sb = [Pv.tile([128, HL * 66 + 64], bf, tag="v", name=f"v{i}")
                for i in range(NKC)]
        qT = [Pq.tile([128, S], bf, tag="qT", name=f"qT{i}") for i in range(NLC)]
        kT = [Pk.tile([128, S], bf, tag="kT", name=f"kT{i}") for i in range(NLC)]
        aoT = [Pa.tile([128, S], bf, tag="aoT", name=f"aoT{i}")
               for i in range(NLC)]

        PSB = _pool("psm", 2, space="PSUM")
        PSe = _pool("pse", 2, space="PSUM")
        PSo = _pool("pso", 1, space="PSUM")
        if True:

            def q_chunk(hp, blk):
                # qT[hp][:, blk*512:+512] = (x @ Wq-half)^T chunk + bias
                ps = PSB.tile([128, 512], f32, tag="psb", name=f"psq{hp}_{blk}")
                for kc in range(NDC):
                    nc.tensor.matmul(
                        out=ps[:],
                        lhsT=wq_sb[kc][:, hp * 128:(hp + 1) * 128],
                        rhs=x_sb[kc][:, blk * 512:(blk + 1) * 512],
                        start=(kc == 0), stop=(kc == NDC - 1))
                nc.scalar.activation(
                    out=qT[hp][:, blk * 512:(blk + 1) * 512],
                    in_=ps[:], func=Ident, bias=bq_sb[:, hp:hp + 1])

            def k_chunk(hp, blk):
                ps = PSB.tile([128, 512], f32, tag="psb", name=f"psk{hp}_{blk}")
                for kc in range(NDC):
                    nc.tensor.matmul(
                        out=ps[:],
                        lhsT=wk_sb[kc][:, hp * 128:(hp + 1) * 128],
                        rhs=x_sb[kc][:, blk * 512:(blk + 1) * 512],
                        start=(kc == 0), stop=(kc == NDC - 1))
                nc.scalar.activation(
                    out=kT[hp][:, blk * 512:(blk + 1) * 512],
                    in_=ps[:], func=Ident, bias=bk_sb[:, hp:hp + 1])

            def v_chunk(sc):
                # v rows for seq chunk sc, all 8 local heads (+ ones col)
                ps = PSB.tile([128, 512], f32, tag="psb", name=f"psv{sc}")
                for dc in range(NDC):
                    nc.tensor.matmul(
                        out=ps[:],
                        lhsT=x_sb[dc][:, sc * 128:(sc + 1) * 128],
                        rhs=wv_sb[dc][:],
                        start=(dc == 0), stop=(dc == NDC - 1))
                v3 = v_sb[sc][:, 0:HL * 66].rearrange("p (h d) -> p h d", d=66)
                nc.gpsimd.memset(v3[:, :, 64:65], 1.0)
                nc.gpsimd.memset(v_sb[sc][:, HL * 66:], 0.0)
                nc.scalar.activation(
                    out=v3[:, :, 0:64],
                    in_=ps[:].rearrange("p (h d) -> p h d", d=64),
                    func=Copy)

            def out_chunk(sc, blk):
                # partial out rows sc*128..+128, cols blk*512..+512
                pf = PSB.tile([128, 512], f32, tag="psb", name=f"pf{sc}_{blk}")
                for dc in range(NLC):
                    nc.tensor.matmul(
                        out=pf[:],
                        lhsT=aoT[dc][:, sc * 128:(sc + 1) * 128],
                        rhs=wo_sb[dc][:, blk * 512:(blk + 1) * 512],
                        start=(dc == 0), stop=(dc == NLC - 1))
                osb = Po.tile([128, 512], f32, tag="osb", name=f"osb{sc}_{blk}")
                nc.scalar.activation(out=osb[:], in_=pf[:], func=Copy)
                nc.sync.dma_start(
                    out=out[sc * 128:(sc + 1) * 128,
                            blk * 512:(blk + 1) * 512],
                    in_=osb[:])

            # drip-feed queue: thunks of ~2us tensor work popped inside the
            # attention loop at a controlled per-kpp rate. Order matters: a
            # thunk must be emitted no later than the kpp whose energy/attnV
            # matmuls consume its output (engine queues run in emission
            # order, so a consumer emitted before its producer deadlocks).
            work = []

            def attention(qb, hp, pre=(), mid=()):
                # pre: thunks the call's first energy depends on (Q/K chunks)
                # mid: safe filler popped at the stt-wait point of each kpp.
                # attnV runs one kpp behind exp so it never waits on it.
                for t in pre:
                    t()
                mid = list(mid)
                rate = -(-len(mid) // 4) if mid else 0
                o_ps = [PSo.tile([128, 512], f32, tag=f"o{i}",
                                 name=f"o{i}_{qb}_{hp}") for i in (0, 1)]
                ex_q = []

                def attnv(kpp, ex4):
                    for jj in range(4):
                        kc = kpp * 4 + jj
                        for i in range(2):
                            h = hp * 2 + i
                            nc.tensor.matmul(
                                out=o_ps[i][:],
                                lhsT=v_sb[kc][:, h * 66:h * 66 + 128],
                                rhs=ex4[:, jj * 1024 + i * 512:
                                        jj * 1024 + (i + 1) * 512],
                                start=(kc == 0), stop=(kc == NKC - 1))

                for kpp in range(NKC // 4):
                    e4 = Pe.tile([128, 4096], bf, tag="e",
                                 name=f"e{qb}_{hp}_{kpp}")
                    ex4 = Pex.tile([128, 4096], bf, tag="ex",
                                   name=f"ex{qb}_{hp}_{kpp}")
                    for jj in range(4):
                        kc = kpp * 4 + jj
                        pe_ps = PSe.tile([128, 1024], f32, tag="pe",
                                         name=f"pe{qb}_{hp}_{kc}")
                        for i in range(2):
                            nc.tensor.matmul(
                                out=pe_ps[:, i * 512:(i + 1) * 512],
                                lhsT=kT[hp][i * 64:(i + 1) * 64,
                                            kc * 128:(kc + 1) * 128],
                                rhs=qT[hp][i * 64:(i + 1) * 64,
                                           qb * 512:(qb + 1) * 512],
                                start=True, stop=True)
                        rep = (mod_sb[(qb, kc)][:, 0:512]
                               .unsqueeze(1).broadcast_to((128, 2, 512)))
                        nc.vector.scalar_tensor_tensor(
                            out=e4[:, jj * 1024:(jj + 1) * 1024]
                                .rearrange("p (r c) -> p r c", r=2),
                            in0=pe_ps[:].rearrange("p (r c) -> p r c", r=2),
                            scalar=1.0, in1=rep, op0=mult, op1=mult)
                        if jj == 1:
                            for _ in range(rate):
                                if mid:
                                    mid.pop(0)()
                    nc.scalar.activation(out=ex4[:], in_=e4[:], func=Exp)
                    ex_q.append((kpp, ex4))
                    if kpp > 0:
                        attnv(*ex_q.pop(0))
                attnv(*ex_q.pop(0))
                while mid:
                    mid.pop(0)()
                # normalize: sums live in psum row 64 (ones column). recip
                # reads psum directly; partition_broadcast (GpSimd, ~1us)
                # hides behind the other head's work.
                bcs = []
                for i in range(2):
                    su = Prs.tile([1, 512], f32, tag="su",
                                  name=f"su{qb}_{hp}_{i}")
                    rc = Prs.tile([1, 512], f32, tag="rc",
                                  name=f"rc{qb}_{hp}_{i}")
                    bc = Pbc.tile([128, 512], f32, tag=f"bc{i}",
                                  name=f"bc{qb}_{hp}_{i}")
                    nc.scalar.activation(out=su[0:1, :],
                                         in_=o_ps[i][64:65, :], func=Copy)
                    nc.vector.reciprocal_approx_fast(out=rc[0:1, :],
                                                     in_=su[0:1, :])
                    nc.gpsimd.partition_broadcast(bc[:], rc[0:1, :])
                    bcs.append(bc)
                for i in range(2):
                    nc.vector.tensor_mul(
                        out=aoT[hp][i * 64:(i + 1) * 64,
                                    qb * 512:(qb + 1) * 512],
                        in0=o_ps[i][0:64, :],
                        in1=bcs[i][i * 64:(i + 1) * 64, :])

            # ---- schedule ----
            def Q(h, b):
                return lambda: q_chunk(h, b)

            def KK(h, b):
                return lambda: k_chunk(h, b)

            def V(s):
                return lambda: v_chunk(s)

            def O(s, b):
                return lambda: out_chunk(s, b)

            # warmup: just enough for attention(0, 0): Q0 blk0, K0 all key
            # blocks, V chunks 0..7. Everything else drips through mid
            # slots of earlier calls, each audited against the kpp deadline
            # of its first consumer.
            q_chunk(0, 0)
            for blk in range(NQB):
                k_chunk(0, blk)
            for sc in range(8):
                v_chunk(sc)

            mids = {
                (0, 0): [V(8), V(9), V(10), V(11), V(12), V(13), V(14),
                         V(15), KK(1, 0), Q(1, 0)],
                (0, 1): [KK(1, 1), KK(1, 2), KK(1, 3), KK(2, 0), Q(2, 0)],
                (0, 2): [KK(2, 1), KK(2, 2), KK(2, 3), KK(3, 0), Q(3, 0)],
                (0, 3): [KK(3, 1), KK(3, 2), KK(3, 3), Q(0, 1)],
            }
            nxt = [(1, 1), (2, 1), (3, 1), (0, 2), (1, 2), (2, 2), (3, 2),
                   (0, 3), (1, 3), (2, 3), (3, 3)]
            for k in range(11):
                qb, hp = divmod(k + 4, NLC)[0], (k + 4) % NLC
                sc = (qb - 1) * 4 + hp
                mids[(qb, hp)] = [O(sc, 0), O(sc, 1), Q(*nxt[k])]
            mids[(3, 3)] = [O(11, 0), O(11, 1)]

            for qb in range(NQB):
                for hp in range(NLC):
                    attention(qb, hp, mid=mids[(qb, hp)])

            # tail: out chunks for qb3 rows.
            for sc in range((NQB - 1) * 4, NQB * 4):
                for blk in range(2):
                    out_chunk(sc, blk)


def build():
    if "nc" in _CACHE:
        return _CACHE["nc"]
    import concourse.bacc as bacc
    import concourse.mybir as mybir
    import concourse.tile as tile

    f32 = mybir.dt.float32
    bf = mybir.dt.bfloat16
    nc = bacc.Bacc("TRN2", target_bir_lowering=False, debug=False,
                   num_devices=N_CORES)
    xT = nc.dram_tensor("xT", [D, S], bf, kind="ExternalInput").ap()
    modT = nc.dram_tensor("modT", [S, S], bf, kind="ExternalInput").ap()
    wq = nc.dram_tensor("wq", [D, DL], bf, kind="ExternalInput").ap()
    wk = nc.dram_tensor("wk", [D, DL], bf, kind="ExternalInput").ap()
    wv = nc.dram_tensor("wv", [D, DL], bf, kind="ExternalInput").ap()
    wo = nc.dram_tensor("wo", [DL, D], bf, kind="ExternalInput").ap()
    bq = nc.dram_tensor("bq", [DL], f32, kind="ExternalInput").ap()
    bk = nc.dram_tensor("bk", [DL], f32, kind="ExternalInput").ap()
    bo = nc.dram_tensor("bo", [D], bf, kind="ExternalInput").ap()
    out = nc.dram_tensor("out", [S, D], f32, kind="ExternalOutput").ap()

    with tile.TileContext(nc) as tc:
        _emit(nc, tc, mybir, (xT, modT, wq, wk, wv, wo, bq, bk, bo, out))
    nc.compile()
    _CACHE["nc"] = nc
    return nc


def prep_inputs(x, key_padding_mask, attn_mask_modifier, Wq, bq, Wk, bk,
                Wv, bv, Wo, bo):
    """Host-side prep -> per-core in_maps (list of 8 dicts)."""
    x = np.asarray(x, np.float32)
    qmask = np.asarray(key_padding_mask, bool)
    mod = np.asarray(attn_mask_modifier, np.float32)
    Wq = np.asarray(Wq, np.float32); bq = np.asarray(bq, np.float32)
    Wk = np.asarray(Wk, np.float32); bk = np.asarray(bk, np.float32)
    Wv = np.asarray(Wv, np.float32); bv = np.asarray(bv, np.float32)
    Wo = np.asarray(Wo, np.float32); bo = np.asarray(bo, np.float32)

    # fold the query-padding mask into the modifier (masked q row -> energy 0
    # -> uniform softmax, identical to the reference's -1e10 fill)
    modm = mod * (~qmask)[:, :, None].astype(np.float32)   # [b, q, k]

    xT_h, modT_h = [], []
    for b in range(B):
        xT_h.append(np.ascontiguousarray(x[b].T).astype(BF))        # [D, S]
        modT_h.append(np.ascontiguousarray(modm[b].T).astype(BF))   # [k, q]

    half = {}
    for c2 in range(2):
        sl = slice(c2 * DL, (c2 + 1) * DL)
        bo_part = np.zeros_like(bo)
        half[c2] = {
            "wq": np.ascontiguousarray(Wq[:, sl] * 0.125).astype(BF),
            "wk": np.ascontiguousarray(Wk[:, sl]).astype(BF),
            "wv": np.ascontiguousarray(Wv[:, sl]).astype(BF),
            "wo": np.ascontiguousarray(Wo[sl, :]).astype(BF),
            "bq": (bq[sl] * 0.125).astype(np.float32),
            "bk": bk[sl].astype(np.float32),
            "bo": bo_part.astype(BF),
        }

    in_maps = []
    for c in range(N_CORES):
        b, c2 = divmod(c, 2)
        m = {"xT": xT_h[b], "modT": modT_h[b]}
        m.update(half[c2])
        in_maps.append(m)
    return in_maps


def assemble(results, bias):
    out = np.empty((B, S, D), np.float32)
    for b in range(B):
        np.add(results[2 * b]["out"], results[2 * b + 1]["out"], out=out[b])
        out[b] += bias
    return out


def full_bias(Wo, bv, bo):
    return (np.asarray(bv, np.float32) @ np.asarray(Wo, np.float32)
            + np.asarray(bo, np.float32))


def kernel(**inputs):
    from concourse.bass_utils import run_bass_kernel_spmd
    nc = build()
    in_maps = prep_inputs(**inputs)
    res = run_bass_kernel_spmd(nc, in_maps, list(range(N_CORES)))
    return assemble(res.results, full_bias(inputs["Wo"], inputs["bv"],
                                           inputs["bo"]))
